# revision 24
# baseline (speedup 1.0000x reference)
"""TLGv4 block-sparse self-attention on 8 trn2 NeuronCores.

Sharding: tensor-parallel over the 8 KV groups (1 group = 4 Q heads + 1 K +
1 V head per core). Each core computes its group's QKV projection columns,
RoPE, block-sparse attention for its 4 Q heads, and a row-sharded partial of
the dense output projection (f16). Host sums the 8 partials (+ b_dense).

Two phases, each tuned to keep the PE stream homogeneous (216ns/matmul):
  1) QKV: wq_g @ hidden^T accumulated in 3 double-buffered PSUM banks per
     512-token slice; bias-add + RoPE on DVE produce qS2/kT2 with rows
     replicated to partitions 64:127 (SBUF->SBUF DMA on the scalar queue)
     so score matmuls for TWO k-chunks run as concurrent PE row-group
     tiles (K=64 each). v^T via XBAR transpose-DMA; 64 ones-columns per
     chunk make the PV matmul emit softmax denominators as ctx rows 0:63.
  2) Attention + dense: per pair, score duos -> one merged [128,1024] Exp
     on ACT (2 PSUM banks) -> block-sparsity masks on GpSimd (diagonal
     chunk first so masks never gate the PV tail) -> PV chain; 1/Z via
     DVE reciprocal_approx_fast from PSUM; 2 merged normalize muls; dense
     trails 3 pairs behind, PSUM -> f16 staging (copies split DVE/ACT) ->
     one 512KB store per pair.
"""
import numpy as np
from contextlib import ExitStack

import concourse.bacc as bacc
import concourse.bass as bass
import concourse.mybir as mybir
import concourse.tile as tile
from concourse.bass_utils import run_bass_kernel_spmd

F32 = mybir.dt.float32
F16 = mybir.dt.float16
AF = mybir.ActivationFunctionType

S = 2048
HID = 2048
D = 64
NQ = 4                      # q heads per kv group
GCOLS = (NQ + 2) * D        # 384 qkv columns per group
NPAIR = S // 128            # 16 pairs of 64-token blocks
SCALE = 1.0 / 8.0           # 1/sqrt(D)
ROPE_BASE = 10000.0
N_CORES = 8


def _pair_chunks(i):
    """128-token k-chunks feeding query pair i (blocks 2i, 2i+1).

    Diagonal chunk first so its causal mask (GpSimd) overlaps the
    remaining score/PV matmuls instead of gating the PV chain tail.
    """
    chunks = [i]
    if i >= 12:
        chunks.append(3)
    chunks += list(range(max(0, i - 8), i))
    return chunks


def _duos(chunks):
    return [tuple(chunks[j:j + 2]) for j in range(0, len(chunks), 2)]


def _build_nc():
    nc = bacc.Bacc()

    ht = nc.declare_dram_parameter("ht", [HID, S], F16, isOutput=False)
    wq = nc.declare_dram_parameter("wq", [128, 16 * GCOLS], F16, isOutput=False)
    bq = nc.declare_dram_parameter("bq", [128, 3], F32, isOutput=False)
    wd = nc.declare_dram_parameter("wd", [128, 2 * HID], F16, isOutput=False)
    cosq = nc.declare_dram_parameter("cosq", [128, S], F16, isOutput=False)
    sinq = nc.declare_dram_parameter("sinq", [128, S], F16, isOutput=False)
    cosk = nc.declare_dram_parameter("cosk", [64, S], F16, isOutput=False)
    sink = nc.declare_dram_parameter("sink", [64, S], F16, isOutput=False)
    tri = nc.declare_dram_parameter("tri", [128, 128], F16, isOutput=False)
    out = nc.declare_dram_parameter("out", [S, HID], F16, isOutput=True)

    with tile.TileContext(nc) as tc, ExitStack() as ctx:
        consts = ctx.enter_context(tc.tile_pool(name="consts", bufs=1))
        persist = ctx.enter_context(tc.tile_pool(name="persist", bufs=1))

        wq_sb = consts.tile([128, 16 * GCOLS], F16)
        wd_sb = consts.tile([128, 2 * HID], F16)
        bq_sb = consts.tile([128, 3], F32)
        cosq_sb = consts.tile([128, S], F16)
        sinq_sb = consts.tile([128, S], F16)
        cosk_sb = consts.tile([64, S], F16)
        sink_sb = consts.tile([64, S], F16)
        tri_sb = consts.tile([128, 128], F16)
        expb = consts.tile([128, 1], F32)
        nc.vector.memset(expb[:], -5.0)

        # persistent activations — per-slice tiles so phase-2 readers only
        # wait on the writes of the slice they actually consume
        qkv = [persist.tile([128, S], F16, tag=f"qkv{m}", name=f"qkv{m}")
               for m in range(3)]
        qSn = [persist.tile([128, NQ * 512], F16, tag=f"qS{n}", name=f"qS{n}")
               for n in range(4)]                # [dup(d), pp*512 + h*128 + t]
        kTn = [persist.tile([128, 512], F16, tag=f"kT{n}", name=f"kT{n}")
               for n in range(4)]                # [dup(d), t]
        vn = [persist.tile([128, 4 * 128], F16, tag=f"v{n}", name=f"v{n}")
              for n in range(4)]                 # [t, cc*128 + (ones | d)]
        ctx_sb = persist.tile([128, 2 * S], F16)   # [(h%2)*64+d, (h//2)*2048+t]

        # ones in cols 0:64 of each chunk: PV emits Z at PSUM rows 0:63
        # (base partition 0 — reciprocal_approx_fast misreads at base 64)
        for n in range(4):
            v_r = vn[n][:].rearrange("p (c w) -> p c w", w=128)
            nc.vector.memset(v_r[:, :, 0:64], 1.0)

        hp = ctx.enter_context(tc.tile_pool(name="hp", bufs=13))
        rp = ctx.enter_context(tc.tile_pool(name="rope", bufs=2))
        exp_p = ctx.enter_context(tc.tile_pool(name="exp", bufs=6))
        rec_p = ctx.enter_context(tc.tile_pool(name="rec", bufs=2))
        stg_p = ctx.enter_context(tc.tile_pool(name="stg", bufs=2))

        def load_ht_slice(n, split_first=False):
            tiles = []
            for q in range(4):
                hq = hp.tile([128, 4 * 512], F16, tag="h", name=f"h{n}_{q}")
                src = ht[q * 512:(q + 1) * 512,
                         n * 512:(n + 1) * 512].rearrange(
                    "(c p) t -> p c t", p=128)
                dst = hq[:].rearrange("p (c t) -> p c t", c=4)
                if q == 0 and split_first:
                    # first matmul only needs the first 128-row chunk
                    nc.sync.dma_start(out=dst[:, 0:1], in_=src[:, 0:1])
                    nc.sync.dma_start(out=dst[:, 1:4], in_=src[:, 1:4])
                else:
                    nc.sync.dma_start(out=dst, in_=src)
                tiles.append(hq)
            return tiles

        # prologue: hidden stream (sync queue) first, weights on scalar queue
        h_tiles = {0: load_ht_slice(0, split_first=True)}
        nc.scalar.dma_start(out=wq_sb[:, 0:GCOLS], in_=wq[:, 0:GCOLS])
        for j in range(3):
            lo, hi = (1 + 5 * j) * GCOLS, (6 + 5 * j) * GCOLS
            nc.scalar.dma_start(out=wq_sb[:, lo:hi], in_=wq[:, lo:hi])
        nc.scalar.dma_start(out=bq_sb[:], in_=bq[:, :])
        nc.scalar.dma_start(out=tri_sb[:], in_=tri[:, :])

        def boundary(n, acc):
            """bias-add + rope + v-transpose + dup for finished slice n."""
            nsl = slice(n * 512, (n + 1) * 512)
            for mc in range(3):
                nc.vector.tensor_scalar_add(
                    qkv[mc][:, nsl], acc[mc][:], bq_sb[:, mc:mc + 1])
            for cc in range(4):
                c = 4 * n + cc
                nc.sync.dma_start_transpose(
                    out=vn[n][:, cc * 128 + 64:(cc + 1) * 128],
                    in_=qkv[2][64:128, c * 128:(c + 1) * 128])
            for ti in range(2):
                qt = qkv[ti]
                rot = rp.tile([128, 512], F16, tag="rot", name="rot")
                for blk in range(4):
                    src = (blk ^ 1) * 32
                    nc.vector.tensor_copy(rot[blk * 32:(blk + 1) * 32, :],
                                          qt[src:src + 32, nsl])
                tmp = rp.tile([128, 512], F16, tag="tmp", name="tmp")
                nc.vector.tensor_mul(tmp[:], qt[:, nsl], cosq_sb[:, nsl])
                nc.vector.tensor_mul(rot[:], rot[:], sinq_sb[:, nsl])
                for half in range(2):  # head 2*ti + half
                    h = 2 * ti + half
                    dst = qSn[n][0:64, :].rearrange(
                        "p (pp hh t) -> p pp hh t", hh=NQ, t=128)[:, :, h, :]
                    nc.vector.tensor_add(
                        dst,
                        tmp[half * 64:(half + 1) * 64, :].rearrange(
                            "p (pp t) -> p pp t", t=128),
                        rot[half * 64:(half + 1) * 64, :].rearrange(
                            "p (pp t) -> p pp t", t=128))
            rotk = rp.tile([128, 512], F16, tag="rot", name="rotk")
            nc.vector.tensor_copy(rotk[0:32, :], qkv[2][32:64, nsl])
            nc.vector.tensor_copy(rotk[32:64, :], qkv[2][0:32, nsl])
            tmpk = rp.tile([128, 512], F16, tag="tmp", name="tmpk")
            nc.vector.tensor_mul(tmpk[0:64, :], qkv[2][0:64, nsl],
                                 cosk_sb[:, nsl])
            nc.vector.tensor_mul(rotk[0:64, :], rotk[0:64, :], sink_sb[:, nsl])
            nc.vector.tensor_add(kTn[n][0:64, :], tmpk[0:64, :], rotk[0:64, :])
            # replicate to partitions 64:127 for row-tiled score matmuls
            nc.scalar.dma_start(out=qSn[n][64:128, :], in_=qSn[n][0:64, :])
            nc.scalar.dma_start(out=kTn[n][64:128, :], in_=kTn[n][0:64, :])

        # ---- phase 1: QKV projection (homogeneous PE stream) ----
        with tc.tile_pool(name="psq", bufs=2, space="PSUM") as psq:
            for n in range(4):
                acc = [psq.tile([128, 512], F32, tag=f"a{m}", name=f"acc{m}")
                       for m in range(3)]
                for kc in range(16):
                    for mc in range(3):
                        nc.tensor.matmul(
                            acc[mc][:],
                            wq_sb[:, kc * GCOLS + mc * 128:
                                  kc * GCOLS + (mc + 1) * 128],
                            h_tiles[n][kc // 4][:, (kc % 4) * 512:
                                                (kc % 4 + 1) * 512],
                            start=(kc == 0), stop=(kc == 15))
                if n == 0:
                    # consts after slice-0 compute is dispatched: they ride
                    # behind the hidden stream instead of starving it
                    h_tiles[1] = load_ht_slice(1)
                    for t_, src_ in ((cosq_sb, cosq), (sinq_sb, sinq),
                                     (cosk_sb, cosk), (sink_sb, sink)):
                        nc.scalar.dma_start(out=t_[:], in_=src_[:, :])
                    h_tiles[2] = load_ht_slice(2)
                if n == 1:
                    h_tiles[3] = load_ht_slice(3)
                    nc.scalar.dma_start(out=wd_sb[:, 0:HID], in_=wd[:, 0:HID])
                    nc.scalar.dma_start(out=wd_sb[:, HID:2 * HID],
                                        in_=wd[:, HID:2 * HID])
                boundary(n, acc)

        # ---- phase 2: attention pairs + trailing dense ----
        tb = tri_sb[:]
        tri_b = bass.AP(tensor=tb.tensor, offset=tb.offset,
                        ap=[tb.ap[0], [0, NQ]] + list(tb.ap[1:]))

        def mask_ex(i, c, exs):
            """block-sparsity masks on an exp'd [128,512] chunk."""
            if c == i:  # diagonal: causal mask, tri broadcast over 4 heads
                exr = exs.rearrange("p (hh t) -> p hh t", hh=NQ)
                nc.gpsimd.tensor_mul(exr, exr, tri_b)
            elif i >= 8 and c == i - 8:
                nc.gpsimd.memset(exs[0:64, :], 0.0)
                if i % 4 != 3:
                    exr = exs[64:128, :].rearrange(
                        "p (hh t) -> p hh t", hh=NQ)
                    nc.gpsimd.memset(exr[:, :, 64:128], 0.0)
            elif i >= 12 and c == 3:
                nc.gpsimd.memset(exs[0:64, :], 0.0)

        with tc.tile_pool(name="duo", bufs=2, space="PSUM") as duo_p, \
             tc.tile_pool(name="psc", bufs=2, space="PSUM") as psc, \
             tc.tile_pool(name="psd", bufs=2, space="PSUM") as psd:

            def emit_pair(i):
                chunks = _pair_chunks(i)
                duos = _duos(chunks)
                ctx_ps = psc.tile([128, 512], F32, name="ctx_ps")
                pv_cnt = [0]
                n_pv = len(chunks)

                def pv(ex, s, c):
                    nc.tensor.matmul(ctx_ps[:],
                                     vn[c // 4][:, (c % 4) * 128:
                                                (c % 4 + 1) * 128],
                                     ex[:, s * 512:(s + 1) * 512],
                                     start=(pv_cnt[0] == 0),
                                     stop=(pv_cnt[0] == n_pv - 1))
                    pv_cnt[0] += 1

                # scores/exp/masks stream duo by duo; PVs trail one duo and
                # the (masked) diagonal chunk's PV is deferred to the end so
                # its mask never gates the accumulation chain
                deferred = []

                def pv_duo(ex, duo):
                    for s, c in enumerate(duo):
                        if c == i:
                            deferred.append((ex, s, c))
                        else:
                            pv(ex, s, c)

                prev = None
                for duo in duos:
                    sps = duo_p.tile([128, 1024], F32, tag="sps", name="sps")
                    for s, c in enumerate(duo):
                        half = slice(s * 64, s * 64 + 64)
                        nc.tensor.matmul(
                            sps[:, s * 512:(s + 1) * 512],
                            kTn[c // 4][half, (c % 4) * 128:
                                        (c % 4 + 1) * 128],
                            qSn[i // 4][half, (i % 4) * 512:
                                        (i % 4 + 1) * 512],
                            start=True, stop=True)
                    ex = exp_p.tile([128, 1024], F16, tag="ex", name="ex")
                    w = 512 * len(duo)
                    nc.scalar.activation(ex[:, 0:w], sps[:, 0:w], AF.Exp,
                                         bias=expb[:])
                    for s, c in enumerate(duo):
                        mask_ex(i, c, ex[:, s * 512:(s + 1) * 512])
                    if prev is not None:
                        pv_duo(*prev)
                    prev = (ex, duo)
                pv_duo(*prev)
                for it in deferred:
                    pv(*it)
                # normalize: rows 0:63 of ctx_ps hold the denominators;
                # two muls, each covering head pair (h, h+2) via panel APs
                rec = rec_p.tile([64, 512], F32, tag="rec", name="rec")
                nc.vector.reciprocal_approx_fast(rec[:], ctx_ps[0:64, :])
                src = ctx_ps[64:128, :].rearrange("p (hh t) -> p hh t", hh=NQ)
                recr = rec[:].rearrange("p (hh t) -> p hh t", hh=NQ)
                for lo in range(2):  # heads (lo, lo+2)
                    dst = ctx_sb[lo * 64:lo * 64 + 64, :].rearrange(
                        "p (pan t) -> p pan t", pan=2)[:, :, i * 128:(i + 1) * 128]
                    nc.vector.tensor_mul(dst, src[:, lo::2, :],
                                         recr[:, lo::2, :])

            def emit_dense(i):
                stg = stg_p.tile([128, HID], F16, tag="stg", name="stg")
                for nn in range(4):
                    dps = psd.tile([128, 512], F32, tag="dps", name="dps")
                    nc.tensor.matmul(dps[:], ctx_sb[:, i * 128:(i + 1) * 128],
                                     wd_sb[:, nn * 512:(nn + 1) * 512],
                                     start=True, stop=False)
                    nc.tensor.matmul(dps[:],
                                     ctx_sb[:, S + i * 128: S + (i + 1) * 128],
                                     wd_sb[:, HID + nn * 512:
                                           HID + (nn + 1) * 512],
                                     start=False, stop=True)
                    if nn == 1:
                        nc.scalar.copy(stg[:, nn * 512:(nn + 1) * 512], dps[:])
                    else:
                        nc.vector.tensor_copy(
                            stg[:, nn * 512:(nn + 1) * 512], dps[:])
                nc.sync.dma_start(out=out[i * 128:(i + 1) * 128, :],
                                  in_=stg[:])

            # big slice-2 pairs first: their inputs (slices <= 2) are ready
            # before rope/dup of slice 3 land, hiding the phase boundary;
            # tiny pairs drain last under the dense backlog
            order = [8, 9, 10, 11, 12, 13, 14, 15,
                     7, 6, 5, 4, 3, 2, 1, 0]
            for idx, i in enumerate(order):
                emit_pair(i)
                if idx >= 3:
                    emit_dense(order[idx - 3])
            for i in order[-3:]:
                emit_dense(i)

    nc.finalize()
    return nc


_NC_CACHE = {}


def _get_nc():
    if "nc" not in _NC_CACHE:
        _NC_CACHE["nc"] = _build_nc()
    return _NC_CACHE["nc"]


def _host_inputs(hidden_states, w_qkv, b_qkv, w_dense):
    h = np.asarray(hidden_states, dtype=np.float32).reshape(S, HID)
    w_qkv = np.asarray(w_qkv, dtype=np.float32)
    b_qkv = np.asarray(b_qkv, dtype=np.float32)
    w_dense = np.asarray(w_dense, dtype=np.float32)

    ht = np.ascontiguousarray(h.T).astype(np.float16)

    inv = 1.0 / (ROPE_BASE ** (np.arange(0, D, 2, dtype=np.float32) / D))
    ang = np.arange(S, dtype=np.float32)[:, None] * inv[None, :]   # [S, 32]
    cosT = np.ascontiguousarray(np.cos(ang).T.astype(np.float32))  # [32, S]
    sinT = np.ascontiguousarray(np.sin(ang).T.astype(np.float32))
    cosq = (np.tile(cosT, (4, 1)) * SCALE).astype(np.float16)
    sinq = (np.concatenate([-sinT, sinT, -sinT, sinT], 0) * SCALE).astype(np.float16)
    cosk = np.tile(cosT, (2, 1)).astype(np.float16)
    sink = np.concatenate([-sinT, sinT], 0).astype(np.float16)

    tri = np.triu(np.ones((128, 128), np.float16))

    in_maps = []
    for g in range(N_CORES):
        wqg = w_qkv[g * GCOLS:(g + 1) * GCOLS, :].T          # [HID, 384]
        wq_t = np.ascontiguousarray(
            wqg.reshape(16, 128, GCOLS).transpose(1, 0, 2).reshape(128, 16 * GCOLS)).astype(np.float16)
        bqg = np.ascontiguousarray(
            b_qkv[g * GCOLS:(g + 1) * GCOLS].reshape(3, 128).T)
        wdg = w_dense[:, g * NQ * D:(g + 1) * NQ * D].T      # [256, HID]
        wd_t = np.ascontiguousarray(
            wdg.reshape(2, 128, HID).transpose(1, 0, 2).reshape(128, 2 * HID)).astype(np.float16)
        in_maps.append({
            "ht": ht, "wq": wq_t, "bq": bqg, "wd": wd_t,
            "cosq": np.ascontiguousarray(cosq), "sinq": np.ascontiguousarray(sinq),
            "cosk": np.ascontiguousarray(cosk), "sink": np.ascontiguousarray(sink),
            "tri": tri,
        })
    return in_maps


def run_device(hidden_states, w_qkv, b_qkv, w_dense, **run_kwargs):
    nc = _get_nc()
    in_maps = _host_inputs(hidden_states, w_qkv, b_qkv, w_dense)
    return run_bass_kernel_spmd(nc, in_maps, list(range(N_CORES)), **run_kwargs)


def kernel(hidden_states, w_qkv, b_qkv, w_dense, b_dense):
    res = run_device(hidden_states, w_qkv, b_qkv, w_dense)
    acc = np.zeros((S, HID), dtype=np.float32)
    for r in res.results:
        acc += r["out"].astype(np.float32)
    acc += np.asarray(b_dense, dtype=np.float32)[None, :]
    return acc.reshape(1, S, HID)


# revision 28
# speedup vs baseline: 1.0457x; 1.0457x over previous
"""TLGv4 block-sparse self-attention on 8 trn2 NeuronCores.

Sharding: tensor-parallel over the 8 KV groups (1 group = 4 Q heads + 1 K +
1 V head per core). Each core computes its group's QKV projection columns,
RoPE, block-sparse attention for its 4 Q heads, and a row-sharded partial of
the dense output projection (f16). Host sums the 8 partials (+ b_dense).

Two phases, each tuned to keep the PE stream homogeneous (216ns/matmul):
  1) QKV: wq_g @ hidden^T accumulated in 3 double-buffered PSUM banks per
     512-token slice; bias-add + RoPE on DVE produce qS2/kT2 with rows
     replicated to partitions 64:127 (SBUF->SBUF DMA on the scalar queue)
     so score matmuls for TWO k-chunks run as concurrent PE row-group
     tiles (K=64 each). v^T via XBAR transpose-DMA; 64 ones-columns per
     chunk make the PV matmul emit softmax denominators as ctx rows 0:63.
  2) Attention + dense: per pair, score duos -> one merged [128,1024] Exp
     on ACT (2 PSUM banks) -> block-sparsity masks on GpSimd (diagonal
     chunk first so masks never gate the PV tail) -> PV chain; 1/Z via
     DVE reciprocal_approx_fast from PSUM; 2 merged normalize muls; dense
     trails 3 pairs behind, PSUM -> f16 staging (copies split DVE/ACT) ->
     one 512KB store per pair.
"""
import numpy as np
from contextlib import ExitStack

import concourse.bacc as bacc
import concourse.bass as bass
import concourse.mybir as mybir
import concourse.tile as tile
from concourse.bass_utils import run_bass_kernel_spmd

F32 = mybir.dt.float32
F16 = mybir.dt.float16
AF = mybir.ActivationFunctionType

S = 2048
HID = 2048
D = 64
NQ = 4                      # q heads per kv group
GCOLS = (NQ + 2) * D        # 384 qkv columns per group
NPAIR = S // 128            # 16 pairs of 64-token blocks
SCALE = 1.0 / 8.0           # 1/sqrt(D)
ROPE_BASE = 10000.0
N_CORES = 8


def _pair_chunks(i):
    """128-token k-chunks feeding query pair i (blocks 2i, 2i+1).

    Diagonal chunk first so its causal mask (GpSimd) overlaps the
    remaining score/PV matmuls instead of gating the PV chain tail.
    """
    chunks = [i]
    if i >= 12:
        chunks.append(3)
    chunks += list(range(max(0, i - 8), i))
    return chunks


def _duos(chunks):
    return [tuple(chunks[j:j + 2]) for j in range(0, len(chunks), 2)]


def _build_nc():
    nc = bacc.Bacc()

    ht = nc.declare_dram_parameter("ht", [HID, S], F16, isOutput=False)
    wq = nc.declare_dram_parameter("wq", [128, 16 * GCOLS], F16, isOutput=False)
    bq = nc.declare_dram_parameter("bq", [128, 3], F32, isOutput=False)
    wd = nc.declare_dram_parameter("wd", [128, 2 * HID], F16, isOutput=False)
    cosq = nc.declare_dram_parameter("cosq", [128, S], F16, isOutput=False)
    sinq = nc.declare_dram_parameter("sinq", [128, S], F16, isOutput=False)
    cosk = nc.declare_dram_parameter("cosk", [64, S], F16, isOutput=False)
    sink = nc.declare_dram_parameter("sink", [64, S], F16, isOutput=False)
    tri = nc.declare_dram_parameter("tri", [128, 128], F16, isOutput=False)
    out = nc.declare_dram_parameter("out", [S, HID], F16, isOutput=True)

    with tile.TileContext(nc) as tc, ExitStack() as ctx:
        consts = ctx.enter_context(tc.tile_pool(name="consts", bufs=1))
        persist = ctx.enter_context(tc.tile_pool(name="persist", bufs=1))

        wq_sb = consts.tile([128, 16 * GCOLS], F16)
        wd_sb = consts.tile([128, 2 * HID], F16)
        bq_sb = consts.tile([128, 3], F32)
        cosq_sb = consts.tile([128, S], F16)
        sinq_sb = consts.tile([128, S], F16)
        cosk_sb = consts.tile([64, S], F16)
        sink_sb = consts.tile([64, S], F16)
        tri_sb = consts.tile([128, 128], F16)
        expb = consts.tile([128, 1], F32)
        nc.vector.memset(expb[:], -5.0)

        # persistent activations — per-slice tiles so phase-2 readers only
        # wait on the writes of the slice they actually consume
        qkv = [persist.tile([128, S], F16, tag=f"qkv{m}", name=f"qkv{m}")
               for m in range(3)]
        qSn = [persist.tile([128, NQ * 512], F16, tag=f"qS{n}", name=f"qS{n}")
               for n in range(4)]                # [dup(d), pp*512 + h*128 + t]
        kTn = [persist.tile([128, 512], F16, tag=f"kT{n}", name=f"kT{n}")
               for n in range(4)]                # [dup(d), t]
        vn = [persist.tile([128, 4 * 128], F16, tag=f"v{n}", name=f"v{n}")
              for n in range(4)]                 # [t, cc*128 + (ones | d)]
        ctx_sb = persist.tile([128, 2 * S], F16)   # [(h%2)*64+d, (h//2)*2048+t]

        # ones in cols 0:64 of each chunk: PV emits Z at PSUM rows 0:63
        # (base partition 0 — reciprocal_approx_fast misreads at base 64)
        for n in range(4):
            v_r = vn[n][:].rearrange("p (c w) -> p c w", w=128)
            nc.vector.memset(v_r[:, :, 0:64], 1.0)

        hp = ctx.enter_context(tc.tile_pool(name="hp", bufs=13))
        rp = ctx.enter_context(tc.tile_pool(name="rope", bufs=2))
        exp_p = ctx.enter_context(tc.tile_pool(name="exp", bufs=6))
        rec_p = ctx.enter_context(tc.tile_pool(name="rec", bufs=2))
        stg_p = ctx.enter_context(tc.tile_pool(name="stg", bufs=2))

        def load_ht_slice(n, split_first=False):
            tiles = []
            for q in range(4):
                hq = hp.tile([128, 4 * 512], F16, tag="h", name=f"h{n}_{q}")
                src = ht[q * 512:(q + 1) * 512,
                         n * 512:(n + 1) * 512].rearrange(
                    "(c p) t -> p c t", p=128)
                dst = hq[:].rearrange("p (c t) -> p c t", c=4)
                if q == 0 and split_first:
                    # first matmul only needs the first 128-row chunk
                    nc.sync.dma_start(out=dst[:, 0:1], in_=src[:, 0:1])
                    nc.sync.dma_start(out=dst[:, 1:4], in_=src[:, 1:4])
                else:
                    nc.sync.dma_start(out=dst, in_=src)
                tiles.append(hq)
            return tiles

        # prologue: first wq chunk + hidden stream interleaved on the sync
        # queue (scalar-queue first-byte proved slow); small consts on scalar
        nc.sync.dma_start(out=wq_sb[:, 0:GCOLS], in_=wq[:, 0:GCOLS])
        h_tiles = {0: load_ht_slice(0, split_first=True)}
        for j in range(3):
            lo, hi = (1 + 5 * j) * GCOLS, (6 + 5 * j) * GCOLS
            nc.sync.dma_start(out=wq_sb[:, lo:hi], in_=wq[:, lo:hi])
        nc.scalar.dma_start(out=bq_sb[:], in_=bq[:, :])
        nc.scalar.dma_start(out=tri_sb[:], in_=tri[:, :])

        def boundary(n, acc):
            """bias-add + rope + v-transpose + dup for finished slice n."""
            nsl = slice(n * 512, (n + 1) * 512)
            for mc in range(3):
                nc.vector.tensor_scalar_add(
                    qkv[mc][:, nsl], acc[mc][:], bq_sb[:, mc:mc + 1])
            for cc in range(4):
                c = 4 * n + cc
                nc.sync.dma_start_transpose(
                    out=vn[n][:, cc * 128 + 64:(cc + 1) * 128],
                    in_=qkv[2][64:128, c * 128:(c + 1) * 128])
            for ti in range(2):
                qt = qkv[ti]
                rot = rp.tile([128, 512], F16, tag="rot", name="rot")
                for blk in range(4):
                    src = (blk ^ 1) * 32
                    nc.vector.tensor_copy(rot[blk * 32:(blk + 1) * 32, :],
                                          qt[src:src + 32, nsl])
                tmp = rp.tile([128, 512], F16, tag="tmp", name="tmp")
                nc.vector.tensor_mul(tmp[:], qt[:, nsl], cosq_sb[:, nsl])
                nc.vector.tensor_mul(rot[:], rot[:], sinq_sb[:, nsl])
                for half in range(2):  # head 2*ti + half
                    h = 2 * ti + half
                    dst = qSn[n][0:64, :].rearrange(
                        "p (pp hh t) -> p pp hh t", hh=NQ, t=128)[:, :, h, :]
                    nc.vector.tensor_add(
                        dst,
                        tmp[half * 64:(half + 1) * 64, :].rearrange(
                            "p (pp t) -> p pp t", t=128),
                        rot[half * 64:(half + 1) * 64, :].rearrange(
                            "p (pp t) -> p pp t", t=128))
            rotk = rp.tile([128, 512], F16, tag="rot", name="rotk")
            nc.vector.tensor_copy(rotk[0:32, :], qkv[2][32:64, nsl])
            nc.vector.tensor_copy(rotk[32:64, :], qkv[2][0:32, nsl])
            tmpk = rp.tile([128, 512], F16, tag="tmp", name="tmpk")
            nc.vector.tensor_mul(tmpk[0:64, :], qkv[2][0:64, nsl],
                                 cosk_sb[:, nsl])
            nc.vector.tensor_mul(rotk[0:64, :], rotk[0:64, :], sink_sb[:, nsl])
            nc.vector.tensor_add(kTn[n][0:64, :], tmpk[0:64, :], rotk[0:64, :])
            # replicate to partitions 64:127 for row-tiled score matmuls.
            # On the sync queue: a dup waiting on rope here must not block
            # the scalar queue, whose next dispatches gate phase-2 exps.
            nc.sync.dma_start(out=qSn[n][64:128, :], in_=qSn[n][0:64, :])
            nc.sync.dma_start(out=kTn[n][64:128, :], in_=kTn[n][0:64, :])

        # ---- phase 1: QKV projection (homogeneous PE stream) ----
        with tc.tile_pool(name="psq", bufs=2, space="PSUM") as psq:
            for n in range(4):
                acc = [psq.tile([128, 512], F32, tag=f"a{m}", name=f"acc{m}")
                       for m in range(3)]
                for kc in range(16):
                    for mc in range(3):
                        nc.tensor.matmul(
                            acc[mc][:],
                            wq_sb[:, kc * GCOLS + mc * 128:
                                  kc * GCOLS + (mc + 1) * 128],
                            h_tiles[n][kc // 4][:, (kc % 4) * 512:
                                                (kc % 4 + 1) * 512],
                            start=(kc == 0), stop=(kc == 15))
                if n == 0:
                    # consts after slice-0 compute is dispatched: they ride
                    # behind the hidden stream instead of starving it
                    h_tiles[1] = load_ht_slice(1)
                    for t_, src_ in ((cosq_sb, cosq), (sinq_sb, sinq),
                                     (cosk_sb, cosk), (sink_sb, sink)):
                        nc.scalar.dma_start(out=t_[:, 0:512],
                                            in_=src_[:, 0:512])
                    h_tiles[2] = load_ht_slice(2)
                if n == 1:
                    for t_, src_ in ((cosq_sb, cosq), (sinq_sb, sinq),
                                     (cosk_sb, cosk), (sink_sb, sink)):
                        nc.scalar.dma_start(out=t_[:, 512:S],
                                            in_=src_[:, 512:S])
                    h_tiles[3] = load_ht_slice(3)
                if n == 2:
                    nc.scalar.dma_start(out=wd_sb[:, 0:HID], in_=wd[:, 0:HID])
                    nc.scalar.dma_start(out=wd_sb[:, HID:2 * HID],
                                        in_=wd[:, HID:2 * HID])
                boundary(n, acc)

        # ---- phase 2: attention pairs + trailing dense ----
        tb = tri_sb[:]
        tri_b = bass.AP(tensor=tb.tensor, offset=tb.offset,
                        ap=[tb.ap[0], [0, NQ]] + list(tb.ap[1:]))

        def mask_ex(i, c, exs):
            """block-sparsity masks on an exp'd [128,512] chunk."""
            if c == i:  # diagonal: causal mask, tri broadcast over 4 heads
                exr = exs.rearrange("p (hh t) -> p hh t", hh=NQ)
                nc.gpsimd.tensor_mul(exr, exr, tri_b)
            elif i >= 8 and c == i - 8:
                nc.gpsimd.memset(exs[0:64, :], 0.0)
                if i % 4 != 3:
                    exr = exs[64:128, :].rearrange(
                        "p (hh t) -> p hh t", hh=NQ)
                    nc.gpsimd.memset(exr[:, :, 64:128], 0.0)
            elif i >= 12 and c == 3:
                nc.gpsimd.memset(exs[0:64, :], 0.0)

        with tc.tile_pool(name="duo", bufs=2, space="PSUM") as duo_p, \
             tc.tile_pool(name="psc", bufs=2, space="PSUM") as psc, \
             tc.tile_pool(name="psd", bufs=2, space="PSUM") as psd:

            def emit_pair(i):
                chunks = _pair_chunks(i)
                duos = _duos(chunks)
                ctx_ps = psc.tile([128, 512], F32, name="ctx_ps")
                pv_cnt = [0]
                n_pv = len(chunks)

                def pv(ex, s, c):
                    nc.tensor.matmul(ctx_ps[:],
                                     vn[c // 4][:, (c % 4) * 128:
                                                (c % 4 + 1) * 128],
                                     ex[:, s * 512:(s + 1) * 512],
                                     start=(pv_cnt[0] == 0),
                                     stop=(pv_cnt[0] == n_pv - 1))
                    pv_cnt[0] += 1

                # scores/exp/masks stream duo by duo; PVs trail one duo and
                # the (masked) diagonal chunk's PV is deferred to the end so
                # its mask never gates the accumulation chain
                deferred = []

                def pv_duo(ex, duo):
                    for s, c in enumerate(duo):
                        if c == i:
                            deferred.append((ex, s, c))
                        else:
                            pv(ex, s, c)

                prev = None
                for duo in duos:
                    sps = duo_p.tile([128, 1024], F32, tag="sps", name="sps")
                    for s, c in enumerate(duo):
                        half = slice(s * 64, s * 64 + 64)
                        nc.tensor.matmul(
                            sps[:, s * 512:(s + 1) * 512],
                            kTn[c // 4][half, (c % 4) * 128:
                                        (c % 4 + 1) * 128],
                            qSn[i // 4][half, (i % 4) * 512:
                                        (i % 4 + 1) * 512],
                            start=True, stop=True)
                    ex = exp_p.tile([128, 1024], F16, tag="ex", name="ex")
                    w = 512 * len(duo)
                    nc.scalar.activation(ex[:, 0:w], sps[:, 0:w], AF.Exp,
                                         bias=expb[:])
                    for s, c in enumerate(duo):
                        mask_ex(i, c, ex[:, s * 512:(s + 1) * 512])
                    if prev is not None:
                        pv_duo(*prev)
                    prev = (ex, duo)
                pv_duo(*prev)
                for it in deferred:
                    pv(*it)
                # normalize: rows 0:63 of ctx_ps hold the denominators;
                # two muls, each covering head pair (h, h+2) via panel APs
                rec = rec_p.tile([64, 512], F32, tag="rec", name="rec")
                nc.vector.reciprocal_approx_fast(rec[:], ctx_ps[0:64, :])
                src = ctx_ps[64:128, :].rearrange("p (hh t) -> p hh t", hh=NQ)
                recr = rec[:].rearrange("p (hh t) -> p hh t", hh=NQ)
                for lo in range(2):  # heads (lo, lo+2)
                    dst = ctx_sb[lo * 64:lo * 64 + 64, :].rearrange(
                        "p (pan t) -> p pan t", pan=2)[:, :, i * 128:(i + 1) * 128]
                    nc.vector.tensor_mul(dst, src[:, lo::2, :],
                                         recr[:, lo::2, :])

            def emit_dense(i):
                stg = stg_p.tile([128, HID], F16, tag="stg", name="stg")
                for nn in range(4):
                    dps = psd.tile([128, 512], F32, tag="dps", name="dps")
                    nc.tensor.matmul(dps[:], ctx_sb[:, i * 128:(i + 1) * 128],
                                     wd_sb[:, nn * 512:(nn + 1) * 512],
                                     start=True, stop=False)
                    nc.tensor.matmul(dps[:],
                                     ctx_sb[:, S + i * 128: S + (i + 1) * 128],
                                     wd_sb[:, HID + nn * 512:
                                           HID + (nn + 1) * 512],
                                     start=False, stop=True)
                    if nn == 1:
                        nc.scalar.copy(stg[:, nn * 512:(nn + 1) * 512], dps[:])
                    else:
                        nc.vector.tensor_copy(
                            stg[:, nn * 512:(nn + 1) * 512], dps[:])
                nc.sync.dma_start(out=out[i * 128:(i + 1) * 128, :],
                                  in_=stg[:])

            for i in range(NPAIR):
                emit_pair(i)
                if i >= 3:
                    emit_dense(i - 3)
            for i in range(NPAIR - 3, NPAIR):
                emit_dense(i)

    nc.finalize()
    return nc


_NC_CACHE = {}


def _get_nc():
    if "nc" not in _NC_CACHE:
        _NC_CACHE["nc"] = _build_nc()
    return _NC_CACHE["nc"]


def _host_inputs(hidden_states, w_qkv, b_qkv, w_dense):
    h = np.asarray(hidden_states, dtype=np.float32).reshape(S, HID)
    w_qkv = np.asarray(w_qkv, dtype=np.float32)
    b_qkv = np.asarray(b_qkv, dtype=np.float32)
    w_dense = np.asarray(w_dense, dtype=np.float32)

    ht = np.ascontiguousarray(h.T).astype(np.float16)

    inv = 1.0 / (ROPE_BASE ** (np.arange(0, D, 2, dtype=np.float32) / D))
    ang = np.arange(S, dtype=np.float32)[:, None] * inv[None, :]   # [S, 32]
    cosT = np.ascontiguousarray(np.cos(ang).T.astype(np.float32))  # [32, S]
    sinT = np.ascontiguousarray(np.sin(ang).T.astype(np.float32))
    cosq = (np.tile(cosT, (4, 1)) * SCALE).astype(np.float16)
    sinq = (np.concatenate([-sinT, sinT, -sinT, sinT], 0) * SCALE).astype(np.float16)
    cosk = np.tile(cosT, (2, 1)).astype(np.float16)
    sink = np.concatenate([-sinT, sinT], 0).astype(np.float16)

    tri = np.triu(np.ones((128, 128), np.float16))

    in_maps = []
    for g in range(N_CORES):
        wqg = w_qkv[g * GCOLS:(g + 1) * GCOLS, :].T          # [HID, 384]
        wq_t = np.ascontiguousarray(
            wqg.reshape(16, 128, GCOLS).transpose(1, 0, 2).reshape(128, 16 * GCOLS)).astype(np.float16)
        bqg = np.ascontiguousarray(
            b_qkv[g * GCOLS:(g + 1) * GCOLS].reshape(3, 128).T)
        wdg = w_dense[:, g * NQ * D:(g + 1) * NQ * D].T      # [256, HID]
        wd_t = np.ascontiguousarray(
            wdg.reshape(2, 128, HID).transpose(1, 0, 2).reshape(128, 2 * HID)).astype(np.float16)
        in_maps.append({
            "ht": ht, "wq": wq_t, "bq": bqg, "wd": wd_t,
            "cosq": np.ascontiguousarray(cosq), "sinq": np.ascontiguousarray(sinq),
            "cosk": np.ascontiguousarray(cosk), "sink": np.ascontiguousarray(sink),
            "tri": tri,
        })
    return in_maps


def run_device(hidden_states, w_qkv, b_qkv, w_dense, **run_kwargs):
    nc = _get_nc()
    in_maps = _host_inputs(hidden_states, w_qkv, b_qkv, w_dense)
    return run_bass_kernel_spmd(nc, in_maps, list(range(N_CORES)), **run_kwargs)


def kernel(hidden_states, w_qkv, b_qkv, w_dense, b_dense):
    res = run_device(hidden_states, w_qkv, b_qkv, w_dense)
    acc = np.zeros((S, HID), dtype=np.float32)
    for r in res.results:
        acc += r["out"].astype(np.float32)
    acc += np.asarray(b_dense, dtype=np.float32)[None, :]
    return acc.reshape(1, S, HID)


# revision 30
# speedup vs baseline: 1.0872x; 1.0397x over previous
"""TLGv4 block-sparse self-attention on 8 trn2 NeuronCores.

Sharding: tensor-parallel over the 8 KV groups (1 group = 4 Q heads + 1 K +
1 V head per core). Each core computes its group's QKV projection columns,
RoPE, block-sparse attention for its 4 Q heads, and a row-sharded partial of
the dense output projection (f16). Host sums the 8 partials (+ b_dense).

Two phases, each tuned to keep the PE stream homogeneous (216ns/matmul):
  1) QKV: wq_g @ hidden^T accumulated in 3 double-buffered PSUM banks per
     512-token slice; bias-add + RoPE on DVE produce qS2/kT2 with rows
     replicated to partitions 64:127 (SBUF->SBUF DMA on the scalar queue)
     so score matmuls for TWO k-chunks run as concurrent PE row-group
     tiles (K=64 each). v^T via XBAR transpose-DMA; 64 ones-columns per
     chunk make the PV matmul emit softmax denominators as ctx rows 0:63.
  2) Attention + dense: per pair, score duos -> one merged [128,1024] Exp
     on ACT (2 PSUM banks) -> block-sparsity masks on GpSimd (diagonal
     chunk first so masks never gate the PV tail) -> PV chain; 1/Z via
     DVE reciprocal_approx_fast from PSUM; 2 merged normalize muls; dense
     trails 3 pairs behind, PSUM -> f16 staging (copies split DVE/ACT) ->
     one 512KB store per pair.
"""
import numpy as np
from contextlib import ExitStack

import concourse.bacc as bacc
import concourse.bass as bass
import concourse.mybir as mybir
import concourse.tile as tile
from concourse.bass_utils import run_bass_kernel_spmd

F32 = mybir.dt.float32
F16 = mybir.dt.float16
AF = mybir.ActivationFunctionType

S = 2048
HID = 2048
D = 64
NQ = 4                      # q heads per kv group
GCOLS = (NQ + 2) * D        # 384 qkv columns per group
NPAIR = S // 128            # 16 pairs of 64-token blocks
SCALE = 1.0 / 8.0           # 1/sqrt(D)
ROPE_BASE = 10000.0
N_CORES = 8


def _pair_chunks(i):
    """128-token k-chunks feeding query pair i (blocks 2i, 2i+1).

    Diagonal chunk first so its causal mask (GpSimd) overlaps the
    remaining score/PV matmuls instead of gating the PV chain tail.
    """
    chunks = [i]
    if i >= 12:
        chunks.append(3)
    chunks += list(range(max(0, i - 8), i))
    return chunks


def _duos(chunks):
    return [tuple(chunks[j:j + 2]) for j in range(0, len(chunks), 2)]


def _build_nc():
    nc = bacc.Bacc()

    ht = nc.declare_dram_parameter("ht", [HID, S], F16, isOutput=False)
    wq = nc.declare_dram_parameter("wq", [128, 16 * GCOLS], F16, isOutput=False)
    bq = nc.declare_dram_parameter("bq", [128, 3], F32, isOutput=False)
    wd = nc.declare_dram_parameter("wd", [128, 2 * HID], F16, isOutput=False)
    cosq = nc.declare_dram_parameter("cosq", [128, S], F16, isOutput=False)
    sinq = nc.declare_dram_parameter("sinq", [128, S], F16, isOutput=False)
    cosk = nc.declare_dram_parameter("cosk", [64, S], F16, isOutput=False)
    sink = nc.declare_dram_parameter("sink", [64, S], F16, isOutput=False)
    tri = nc.declare_dram_parameter("tri", [128, 128], F16, isOutput=False)
    out = nc.declare_dram_parameter("out", [S, HID], F16, isOutput=True)

    with tile.TileContext(nc) as tc, ExitStack() as ctx:
        consts = ctx.enter_context(tc.tile_pool(name="consts", bufs=1))
        persist = ctx.enter_context(tc.tile_pool(name="persist", bufs=1))

        wq_sb = consts.tile([128, 16 * GCOLS], F16)
        wd_sb = consts.tile([128, 2 * HID], F16)
        bq_sb = consts.tile([128, 3], F32)
        cosq_sb = consts.tile([128, S], F16)
        sinq_sb = consts.tile([128, S], F16)
        cosk_sb = consts.tile([64, S], F16)
        sink_sb = consts.tile([64, S], F16)
        tri_sb = consts.tile([128, 128], F16)
        expb = consts.tile([128, 1], F32)
        nc.vector.memset(expb[:], -5.0)

        # persistent activations — per-slice tiles so phase-2 readers only
        # wait on the writes of the slice they actually consume
        qkv = [persist.tile([128, S], F16, tag=f"qkv{m}", name=f"qkv{m}")
               for m in range(3)]
        qSn = [persist.tile([128, NQ * 512], F16, tag=f"qS{n}", name=f"qS{n}")
               for n in range(4)]                # [dup(d), pp*512 + h*128 + t]
        kTn = [persist.tile([128, 512], F16, tag=f"kT{n}", name=f"kT{n}")
               for n in range(4)]                # [dup(d), t]
        vn = [persist.tile([128, 4 * 128], F16, tag=f"v{n}", name=f"v{n}")
              for n in range(4)]                 # [t, cc*128 + (ones | d)]
        ctx_sb = persist.tile([128, 2 * S], F16)   # [(h%2)*64+d, (h//2)*2048+t]

        # ones in cols 0:64 of each chunk: PV emits Z at PSUM rows 0:63
        # (base partition 0 — reciprocal_approx_fast misreads at base 64)
        for n in range(4):
            v_r = vn[n][:].rearrange("p (c w) -> p c w", w=128)
            nc.vector.memset(v_r[:, :, 0:64], 1.0)

        hp = ctx.enter_context(tc.tile_pool(name="hp", bufs=13))
        rp = ctx.enter_context(tc.tile_pool(name="rope", bufs=2))
        exp_p = ctx.enter_context(tc.tile_pool(name="exp", bufs=6))
        rec_p = ctx.enter_context(tc.tile_pool(name="rec", bufs=2))
        stg_p = ctx.enter_context(tc.tile_pool(name="stg", bufs=2))

        def load_ht_slice(n, split_first=False):
            # a single DMA queue sustains only ~100GB/s: alternate the
            # hidden-state quarters across the sync and scalar HWDGE queues
            tiles = []
            for q in range(4):
                eng = nc.sync if q % 2 == 0 else nc.scalar
                hq = hp.tile([128, 4 * 512], F16, tag="h", name=f"h{n}_{q}")
                src = ht[q * 512:(q + 1) * 512,
                         n * 512:(n + 1) * 512].rearrange(
                    "(c p) t -> p c t", p=128)
                dst = hq[:].rearrange("p (c t) -> p c t", c=4)
                if q == 0 and split_first:
                    # first matmul only needs the first 128-row chunk
                    eng.dma_start(out=dst[:, 0:1], in_=src[:, 0:1])
                    eng.dma_start(out=dst[:, 1:4], in_=src[:, 1:4])
                else:
                    eng.dma_start(out=dst, in_=src)
                tiles.append(hq)
            return tiles

        # prologue: qkv weights + consts ride the GpSimd SWDGE queue so the
        # two HWDGE queues are dedicated to the hidden-state stream
        nc.gpsimd.dma_start(out=wq_sb[:, 0:GCOLS], in_=wq[:, 0:GCOLS])
        h_tiles = {0: load_ht_slice(0, split_first=True)}
        for j in range(3):
            lo, hi = (1 + 5 * j) * GCOLS, (6 + 5 * j) * GCOLS
            nc.gpsimd.dma_start(out=wq_sb[:, lo:hi], in_=wq[:, lo:hi])
        nc.gpsimd.dma_start(out=bq_sb[:], in_=bq[:, :])
        nc.gpsimd.dma_start(out=tri_sb[:], in_=tri[:, :])

        def boundary(n, acc):
            """bias-add + rope + v-transpose + dup for finished slice n."""
            nsl = slice(n * 512, (n + 1) * 512)
            for mc in range(3):
                nc.vector.tensor_scalar_add(
                    qkv[mc][:, nsl], acc[mc][:], bq_sb[:, mc:mc + 1])
            for cc in range(4):
                c = 4 * n + cc
                nc.sync.dma_start_transpose(
                    out=vn[n][:, cc * 128 + 64:(cc + 1) * 128],
                    in_=qkv[2][64:128, c * 128:(c + 1) * 128])
            for ti in range(2):
                qt = qkv[ti]
                rot = rp.tile([128, 512], F16, tag="rot", name="rot")
                for blk in range(4):
                    src = (blk ^ 1) * 32
                    nc.vector.tensor_copy(rot[blk * 32:(blk + 1) * 32, :],
                                          qt[src:src + 32, nsl])
                tmp = rp.tile([128, 512], F16, tag="tmp", name="tmp")
                nc.vector.tensor_mul(tmp[:], qt[:, nsl], cosq_sb[:, nsl])
                nc.vector.tensor_mul(rot[:], rot[:], sinq_sb[:, nsl])
                for half in range(2):  # head 2*ti + half
                    h = 2 * ti + half
                    dst = qSn[n][0:64, :].rearrange(
                        "p (pp hh t) -> p pp hh t", hh=NQ, t=128)[:, :, h, :]
                    nc.vector.tensor_add(
                        dst,
                        tmp[half * 64:(half + 1) * 64, :].rearrange(
                            "p (pp t) -> p pp t", t=128),
                        rot[half * 64:(half + 1) * 64, :].rearrange(
                            "p (pp t) -> p pp t", t=128))
            rotk = rp.tile([128, 512], F16, tag="rot", name="rotk")
            nc.vector.tensor_copy(rotk[0:32, :], qkv[2][32:64, nsl])
            nc.vector.tensor_copy(rotk[32:64, :], qkv[2][0:32, nsl])
            tmpk = rp.tile([128, 512], F16, tag="tmp", name="tmpk")
            nc.vector.tensor_mul(tmpk[0:64, :], qkv[2][0:64, nsl],
                                 cosk_sb[:, nsl])
            nc.vector.tensor_mul(rotk[0:64, :], rotk[0:64, :], sink_sb[:, nsl])
            nc.vector.tensor_add(kTn[n][0:64, :], tmpk[0:64, :], rotk[0:64, :])
            # replicate to partitions 64:127 for row-tiled score matmuls.
            # On the sync queue: a dup waiting on rope here must not block
            # the scalar queue, whose next dispatches gate phase-2 exps.
            nc.sync.dma_start(out=qSn[n][64:128, :], in_=qSn[n][0:64, :])
            nc.sync.dma_start(out=kTn[n][64:128, :], in_=kTn[n][0:64, :])

        # ---- phase 1: QKV projection (homogeneous PE stream) ----
        with tc.tile_pool(name="psq", bufs=2, space="PSUM") as psq:
            for n in range(4):
                acc = [psq.tile([128, 512], F32, tag=f"a{m}", name=f"acc{m}")
                       for m in range(3)]
                for kc in range(16):
                    for mc in range(3):
                        nc.tensor.matmul(
                            acc[mc][:],
                            wq_sb[:, kc * GCOLS + mc * 128:
                                  kc * GCOLS + (mc + 1) * 128],
                            h_tiles[n][kc // 4][:, (kc % 4) * 512:
                                                (kc % 4 + 1) * 512],
                            start=(kc == 0), stop=(kc == 15))
                if n == 0:
                    # consts after slice-0 compute is dispatched: they ride
                    # behind the hidden stream instead of starving it
                    h_tiles[1] = load_ht_slice(1)
                    for t_, src_ in ((cosq_sb, cosq), (sinq_sb, sinq),
                                     (cosk_sb, cosk), (sink_sb, sink)):
                        nc.gpsimd.dma_start(out=t_[:, 0:512],
                                            in_=src_[:, 0:512])
                    h_tiles[2] = load_ht_slice(2)
                if n == 1:
                    for t_, src_ in ((cosq_sb, cosq), (sinq_sb, sinq),
                                     (cosk_sb, cosk), (sink_sb, sink)):
                        nc.gpsimd.dma_start(out=t_[:, 512:S],
                                            in_=src_[:, 512:S])
                    h_tiles[3] = load_ht_slice(3)
                if n == 2:
                    nc.gpsimd.dma_start(out=wd_sb[:, 0:HID], in_=wd[:, 0:HID])
                    nc.gpsimd.dma_start(out=wd_sb[:, HID:2 * HID],
                                        in_=wd[:, HID:2 * HID])
                boundary(n, acc)

        # ---- phase 2: attention pairs + trailing dense ----
        tb = tri_sb[:]
        tri_b = bass.AP(tensor=tb.tensor, offset=tb.offset,
                        ap=[tb.ap[0], [0, NQ]] + list(tb.ap[1:]))

        def mask_ex(i, c, exs):
            """block-sparsity masks on an exp'd [128,512] chunk."""
            if c == i:  # diagonal: causal mask, tri broadcast over 4 heads
                exr = exs.rearrange("p (hh t) -> p hh t", hh=NQ)
                nc.gpsimd.tensor_mul(exr, exr, tri_b)
            elif i >= 8 and c == i - 8:
                nc.gpsimd.memset(exs[0:64, :], 0.0)
                if i % 4 != 3:
                    exr = exs[64:128, :].rearrange(
                        "p (hh t) -> p hh t", hh=NQ)
                    nc.gpsimd.memset(exr[:, :, 64:128], 0.0)
            elif i >= 12 and c == 3:
                nc.gpsimd.memset(exs[0:64, :], 0.0)

        with tc.tile_pool(name="duo", bufs=2, space="PSUM") as duo_p, \
             tc.tile_pool(name="psc", bufs=2, space="PSUM") as psc, \
             tc.tile_pool(name="psd", bufs=2, space="PSUM") as psd:

            def emit_pair(i):
                chunks = _pair_chunks(i)
                duos = _duos(chunks)
                ctx_ps = psc.tile([128, 512], F32, name="ctx_ps")
                pv_cnt = [0]
                n_pv = len(chunks)

                def pv(ex, s, c):
                    nc.tensor.matmul(ctx_ps[:],
                                     vn[c // 4][:, (c % 4) * 128:
                                                (c % 4 + 1) * 128],
                                     ex[:, s * 512:(s + 1) * 512],
                                     start=(pv_cnt[0] == 0),
                                     stop=(pv_cnt[0] == n_pv - 1))
                    pv_cnt[0] += 1

                # scores/exp/masks stream duo by duo; PVs trail one duo and
                # the (masked) diagonal chunk's PV is deferred to the end so
                # its mask never gates the accumulation chain
                deferred = []

                def pv_duo(ex, duo):
                    for s, c in enumerate(duo):
                        if c == i:
                            deferred.append((ex, s, c))
                        else:
                            pv(ex, s, c)

                prev = None
                for duo in duos:
                    sps = duo_p.tile([128, 1024], F32, tag="sps", name="sps")
                    for s, c in enumerate(duo):
                        half = slice(s * 64, s * 64 + 64)
                        nc.tensor.matmul(
                            sps[:, s * 512:(s + 1) * 512],
                            kTn[c // 4][half, (c % 4) * 128:
                                        (c % 4 + 1) * 128],
                            qSn[i // 4][half, (i % 4) * 512:
                                        (i % 4 + 1) * 512],
                            start=True, stop=True)
                    ex = exp_p.tile([128, 1024], F16, tag="ex", name="ex")
                    w = 512 * len(duo)
                    nc.scalar.activation(ex[:, 0:w], sps[:, 0:w], AF.Exp,
                                         bias=expb[:])
                    for s, c in enumerate(duo):
                        mask_ex(i, c, ex[:, s * 512:(s + 1) * 512])
                    if prev is not None:
                        pv_duo(*prev)
                    prev = (ex, duo)
                pv_duo(*prev)
                for it in deferred:
                    pv(*it)
                # normalize: rows 0:63 of ctx_ps hold the denominators;
                # two muls, each covering head pair (h, h+2) via panel APs
                rec = rec_p.tile([64, 512], F32, tag="rec", name="rec")
                nc.vector.reciprocal_approx_fast(rec[:], ctx_ps[0:64, :])
                src = ctx_ps[64:128, :].rearrange("p (hh t) -> p hh t", hh=NQ)
                recr = rec[:].rearrange("p (hh t) -> p hh t", hh=NQ)
                for lo in range(2):  # heads (lo, lo+2)
                    dst = ctx_sb[lo * 64:lo * 64 + 64, :].rearrange(
                        "p (pan t) -> p pan t", pan=2)[:, :, i * 128:(i + 1) * 128]
                    nc.vector.tensor_mul(dst, src[:, lo::2, :],
                                         recr[:, lo::2, :])

            def emit_dense(i):
                stg = stg_p.tile([128, HID], F16, tag="stg", name="stg")
                for nn in range(4):
                    dps = psd.tile([128, 512], F32, tag="dps", name="dps")
                    nc.tensor.matmul(dps[:], ctx_sb[:, i * 128:(i + 1) * 128],
                                     wd_sb[:, nn * 512:(nn + 1) * 512],
                                     start=True, stop=False)
                    nc.tensor.matmul(dps[:],
                                     ctx_sb[:, S + i * 128: S + (i + 1) * 128],
                                     wd_sb[:, HID + nn * 512:
                                           HID + (nn + 1) * 512],
                                     start=False, stop=True)
                    if nn == 1:
                        nc.scalar.copy(stg[:, nn * 512:(nn + 1) * 512], dps[:])
                    else:
                        nc.vector.tensor_copy(
                            stg[:, nn * 512:(nn + 1) * 512], dps[:])
                nc.sync.dma_start(out=out[i * 128:(i + 1) * 128, :],
                                  in_=stg[:])

            for i in range(NPAIR):
                emit_pair(i)
                if i >= 3:
                    emit_dense(i - 3)
            for i in range(NPAIR - 3, NPAIR):
                emit_dense(i)

    nc.finalize()
    return nc


_NC_CACHE = {}


def _get_nc():
    if "nc" not in _NC_CACHE:
        _NC_CACHE["nc"] = _build_nc()
    return _NC_CACHE["nc"]


def _host_inputs(hidden_states, w_qkv, b_qkv, w_dense):
    h = np.asarray(hidden_states, dtype=np.float32).reshape(S, HID)
    w_qkv = np.asarray(w_qkv, dtype=np.float32)
    b_qkv = np.asarray(b_qkv, dtype=np.float32)
    w_dense = np.asarray(w_dense, dtype=np.float32)

    ht = np.ascontiguousarray(h.T).astype(np.float16)

    inv = 1.0 / (ROPE_BASE ** (np.arange(0, D, 2, dtype=np.float32) / D))
    ang = np.arange(S, dtype=np.float32)[:, None] * inv[None, :]   # [S, 32]
    cosT = np.ascontiguousarray(np.cos(ang).T.astype(np.float32))  # [32, S]
    sinT = np.ascontiguousarray(np.sin(ang).T.astype(np.float32))
    cosq = (np.tile(cosT, (4, 1)) * SCALE).astype(np.float16)
    sinq = (np.concatenate([-sinT, sinT, -sinT, sinT], 0) * SCALE).astype(np.float16)
    cosk = np.tile(cosT, (2, 1)).astype(np.float16)
    sink = np.concatenate([-sinT, sinT], 0).astype(np.float16)

    tri = np.triu(np.ones((128, 128), np.float16))

    in_maps = []
    for g in range(N_CORES):
        wqg = w_qkv[g * GCOLS:(g + 1) * GCOLS, :].T          # [HID, 384]
        wq_t = np.ascontiguousarray(
            wqg.reshape(16, 128, GCOLS).transpose(1, 0, 2).reshape(128, 16 * GCOLS)).astype(np.float16)
        bqg = np.ascontiguousarray(
            b_qkv[g * GCOLS:(g + 1) * GCOLS].reshape(3, 128).T)
        wdg = w_dense[:, g * NQ * D:(g + 1) * NQ * D].T      # [256, HID]
        wd_t = np.ascontiguousarray(
            wdg.reshape(2, 128, HID).transpose(1, 0, 2).reshape(128, 2 * HID)).astype(np.float16)
        in_maps.append({
            "ht": ht, "wq": wq_t, "bq": bqg, "wd": wd_t,
            "cosq": np.ascontiguousarray(cosq), "sinq": np.ascontiguousarray(sinq),
            "cosk": np.ascontiguousarray(cosk), "sink": np.ascontiguousarray(sink),
            "tri": tri,
        })
    return in_maps


def run_device(hidden_states, w_qkv, b_qkv, w_dense, **run_kwargs):
    nc = _get_nc()
    in_maps = _host_inputs(hidden_states, w_qkv, b_qkv, w_dense)
    return run_bass_kernel_spmd(nc, in_maps, list(range(N_CORES)), **run_kwargs)


def kernel(hidden_states, w_qkv, b_qkv, w_dense, b_dense):
    res = run_device(hidden_states, w_qkv, b_qkv, w_dense)
    acc = np.zeros((S, HID), dtype=np.float32)
    for r in res.results:
        acc += r["out"].astype(np.float32)
    acc += np.asarray(b_dense, dtype=np.float32)[None, :]
    return acc.reshape(1, S, HID)
